# revision 45
# baseline (speedup 1.0000x reference)
"""BEV-pool (lift-splat-shoot scatter-sum) Trainium2 Bass kernel, v4.

Pipeline
--------
Host (numpy, index math + quantization only):
  * mirror the reference geometry in float32 to voxelize every frustum
    point (value-identical to the jax/CPU reference on in-bounds points)
  * compact the ~2k occupied voxels into a dense slot space; split each
    voxel's points across the 8 cores with leveled remainders so per-core
    cumulative counts never drift by more than 1: every core's stream is
    its own points packed densely in slot order with NO per-slot padding,
    yet one shared SPMD window schedule covers all cores (per-core lid
    columns carry the differences)
  * quantize the point features to fp8 e3m4 with ERROR FEEDBACK along
    each (core, voxel) chain: q_i = rnd(x_i + carry_{i-1}). The device
    sums each chain exactly in fp32 PSUM, so a voxel partial-sum carries
    only the FINAL carry (~one e3m4 ulp) instead of sqrt(k) accumulated
    ulps -> ~0.33% rel err at HALF the bf16 stream bytes
  * lay each core's points out slot-sorted and chunk-transposed in DRAM
    ([128, nch*80] fp8) so the device streams them with plain wide
    contiguous DMAs at the full DMA-fleet rate; chunk order is
    descending-slot so the final chunks touch only the lowest slots and
    the post-stream writeback tail is minimal
  * bake the per-matmul window-relative slot ids (bf16) into a small
    meta tensor

Device (per core, Bass/Tile):
  * slab0 is issued before the short meta transfer so the DMA engines
    are never idle during the HWDGE issue-chain fill; ~9us of fp8 slab
    transfers then stream back-to-back on the sync-engine HWDGE queue
  * one-hot rhs per matmul, built 32-matmuls-per-DVE-instruction in a
    TRANSPOSED batch layout oh[p, w*KB + m]: every operand's last AP dim
    is packed stride-1 bf16, so the DVE runs its 2x perf mode
    (~10ns/matmul); iota_rep (compare target, value w at w*KB+m) is 16
    DVE memsets during the preamble
  * matmuls use the ACTUAL window span (mean ~7 cols), keeping PE ~2us;
    the first matmul touching each PSUM bank runs start=True, zeroing
    the bank's 2KB zero region (no separate zero-matmul feed)
  * the window schedule is clipped at two forced boundaries fins =
    (max slot of the last slab, max slot of the last two slabs), and
    those two tail column ranges accumulate in their own spare PSUM
    banks: Tile tracks PSUM WAR hazards per bank, so the earlier
    ranges' PSUM-read copies never stall the tail slabs' matmuls
  * writeback overlaps the stream: each column range is copied
    PSUM->SBUF the moment its last matmul retires (scalar engine for
    the bulk ranges, DVE for the two tail ranges so they don't queue
    behind a bulk copy) and DMA'd out — bulk ranges on the Pool SWDGE
    queue (its sem pool is disjoint from the slab DMAs' HWDGE sems),
    the final two on the by-then-idle sync/scalar HWDGE queues
  * a post-pass orders the end-of-program barrier's split sem-wait walk
    by expected resolution time (slab lanes, then writeback lanes in
    completion order), so the walk parks on the true last event instead
    of serializing ~0.7us of satisfied waits behind it

Host combine: sum the 8 aligned [80, 2044] bf16 partials in fp64,
scatter the compact slot rows into the zeros output grid.

A post-pass splits multi-wait instructions into single-wait
EventSemaphores (this walrus build accepts only one sync-wait slot per
instruction struct).
"""

import os
import numpy as np
import ml_dtypes

BF16 = ml_dtypes.bfloat16
FP8 = ml_dtypes.float8_e3m4

# ---- problem constants (from the reference nn.Module) ----
IMAGE_SIZE = (256, 704)
FEATURE_SIZE = (32, 88)
XBOUND = (-54.0, 54.0, 0.3)
YBOUND = (-54.0, 54.0, 0.3)
ZBOUND = (-10.0, 10.0, 20.0)
DBOUND = (1.0, 60.0, 1.0)

N_CORES = 8
P = 128          # points per chunk / matmul contraction dim
OH_W = 16        # one-hot window width (max moving cols per matmul)
BANK_W = 512     # PSUM bank width in fp32
KB = 32          # matmuls per batched DVE is_equal
CS = 32          # chunks per stream slab DMA


def _host_geometry(img_trans, img_scale, lidar2img, B, N, D, H, W):
    """float32 numpy mirror of the reference get_geometry + voxelize."""
    dx = np.array([XBOUND[2], YBOUND[2], ZBOUND[2]], np.float32)
    bx = np.array([XBOUND[0] + XBOUND[2] / 2.0,
                   YBOUND[0] + YBOUND[2] / 2.0,
                   ZBOUND[0] + ZBOUND[2] / 2.0], np.float32)
    nx = [int((b[1] - b[0]) / b[2]) for b in (XBOUND, YBOUND, ZBOUND)]
    NX, NY, NZ = nx

    iH, iW = IMAGE_SIZE
    fH, fW = FEATURE_SIZE
    ds = np.arange(DBOUND[0], DBOUND[1], DBOUND[2], dtype=np.float32)
    xs = np.linspace(0.0, iW - 1, fW, dtype=np.float32)
    ys = np.linspace(0.0, iH - 1, fH, dtype=np.float32)
    assert ds.shape[0] == D and fH == H and fW == W

    fr = np.stack([
        np.broadcast_to(xs[None, None, :], (D, H, W)),
        np.broadcast_to(ys[None, :, None], (D, H, W)),
        np.broadcast_to(ds[:, None, None], (D, H, W)),
    ], axis=-1).astype(np.float32)                       # [D,H,W,3]

    pts = fr[None, None] + img_trans[:, :, None, None, None, :]
    d = pts[..., 2:3]
    xy = pts[..., :2] / img_scale[:, :, None, None, None, None]
    p4 = np.concatenate([xy * d, d, np.ones_like(d)], axis=-1)
    img2lidar = np.linalg.inv(lidar2img)
    geom = np.einsum('bnij,bndhwj->bndhwi', img2lidar, p4)[..., :3]
    geom = geom.astype(np.float32)
    vox = ((geom - (bx - dx / 2.0)) / dx).astype(np.int32)  # trunc toward 0
    mask = ((vox[..., 0] >= 0) & (vox[..., 0] < NX)
            & (vox[..., 1] >= 0) & (vox[..., 1] < NY)
            & (vox[..., 2] >= 0) & (vox[..., 2] < NZ))
    flat = (vox[..., 2] * NX + vox[..., 0]) * NY + vox[..., 1]
    flat = flat + np.arange(B, dtype=np.int32)[:, None, None, None, None] \
        * (NZ * NX * NY)
    flatm = np.where(mask, flat, -1).reshape(-1)
    return flatm, (NX, NY, NZ)


def _pick_fins(slots8, nch, plan):
    """Forced writeback-range boundaries: just above the highest slot the
    last slab (and the last two slabs) touch, so the bulk of the grid
    writes back while the tail slabs' DMA-sem propagations are still in
    flight and only two small copies + DMAs trail the stream."""
    fins = []
    for k in (1, 2):
        w = sum(plan[-k:])
        tail = slots8[:, (nch - w) * P:]
        real = tail[tail >= 0]
        f = int(real.max()) + 1 if len(real) else 1
        fins.append(min(f, BANK_W))
    fins = sorted(set(f for f in fins if 0 < f < BANK_W))
    return fins


def _build_schedule(slots8, nch, S, fins):
    """Per-matmul windows over the per-core slot-sorted point layouts.

    slots8: [N_CORES, nch*P] slot id per stream position (-1 = zero pad).
    The window schedule is shared across cores (SPMD program); each core
    gets its own lid columns. Returns (mm list of (chunk, bank, col_lo,
    w_actual), lids [N_CORES, P, n_mmp] f32, n_mmp, last_mm_of_bank).
    w_actual is the true span of occupied slots in the window, so the
    matmul only streams the PSUM columns that can receive points.
    Windows are clipped at the forced fins (as at bank boundaries) so no
    matmul straddles a writeback range edge.
    """
    mms = []
    lid_cols = [[] for _ in range(N_CORES)]
    for j in range(nch):
        sl8 = slots8[:, j * P:(j + 1) * P]
        real = sl8[sl8 >= 0]
        if len(real) == 0:
            continue
        cur = int(real.min())
        while True:
            bank = cur // BANK_W
            wend = min(cur + OH_W, (bank + 1) * BANK_W, S)
            for f in fins:
                if cur < f:
                    wend = min(wend, f)
                    break
            inw = real[(real >= cur) & (real < wend)]
            w_act = int(inw.max()) - cur + 1
            for k in range(N_CORES):
                lid_cols[k].append(
                    np.where((sl8[k] >= cur) & (sl8[k] < wend),
                             sl8[k] - cur, -1.0).astype(np.float32))
            mms.append((j, bank, cur - bank * BANK_W, w_act))
            nxt = real[real >= wend]
            if len(nxt) == 0:
                break
            cur = int(nxt.min())
    n_mm = len(mms)
    n_mmp = -(-n_mm // KB) * KB
    lids = np.full((N_CORES, P, n_mmp), -1.0, np.float32)
    if n_mm:
        for k in range(N_CORES):
            lids[k, :, :n_mm] = np.stack(lid_cols[k], axis=1)
    last_mm_of_bank = {}
    for i, (j, bank, lo, w) in enumerate(mms):
        last_mm_of_bank[bank] = i
    return mms, lids, n_mmp, last_mm_of_bank


def _wb_ranges(mms, S, fins):
    """Writeback column ranges + the last matmul touching each; ranges are
    written back the moment their last matmul retires. Range edges are
    bank boundaries plus the forced fins."""
    bounds = sorted({0} | {min(f, S) for f in fins} |
                    {b for b in range(BANK_W, S, BANK_W)} | {S})
    ranges = list(zip(bounds[:-1], bounds[1:]))
    last_mm_of_rng = {}
    for i, (j, bank, lo, w) in enumerate(mms):
        c0, c1 = bank * BANK_W + lo, bank * BANK_W + lo + w
        for r, (r0, r1) in enumerate(ranges):
            if c0 < r1 and c1 > r0:
                last_mm_of_rng[r] = i
    by_completion = sorted(last_mm_of_rng, key=lambda r: last_mm_of_rng[r])
    return ranges, last_mm_of_rng, by_completion


def _slab_plan(nch):
    """Chunk counts per slab DMA: CS-sized middles, shrinking tail (short
    post-stream dependency chain; last slab >=12 chunks keeps its
    per-descriptor bytes above the 512B small-transfer penalty)."""
    tail = [c for c in (16, 12) if nch > 48]
    mid = nch - sum(tail)
    plan = [CS] * (mid // CS)
    if mid % CS:
        plan.append(mid % CS)
    plan += tail
    assert sum(plan) == nch and plan
    return plan


def _build_bass(nch, n_mmp, mms, last_mm_of_bank, S, C, fins):
    import concourse.bass as bass
    import concourse.mybir as mybir
    import concourse.tile as tile

    f32 = mybir.dt.float32
    bf16 = mybir.dt.bfloat16
    fp8 = mybir.dt.float8e3
    MW = max(n_mmp, 256)                    # >=512B/partition meta transfer
    plan = _slab_plan(nch)
    slab_of_chunk = np.repeat(np.arange(len(plan)), plan)
    slab_c0 = np.concatenate([[0], np.cumsum(plan)[:-1]])

    ranges, last_mm_of_rng, by_completion = _wb_ranges(mms, S, fins)

    # Tile tracks PSUM WAR hazards at bank granularity: an act-copy
    # reading a bank's finished column range would serialize every LATER
    # matmul into the same bank behind it (+~1us on the tail). Give each
    # forced tail range [0,fins[0]) / [fins[0],fins[1]) its own spare
    # PSUM bank so the tail matmuls and the earlier ranges' reads never
    # share a bank.
    n_grid = -(-S // BANK_W)

    def pbank(c0):
        for i, f in enumerate(fins):
            if c0 < f:
                base = fins[i - 1] if i else 0
                return n_grid + i, c0 - base
        return c0 // BANK_W, c0 % BANK_W

    n_banks = n_grid + len(fins)
    assert n_banks <= 8
    mms = [(j, *pbank(bank * BANK_W + lo), w) for (j, bank, lo, w) in mms]
    first_mm_of_bank = {}
    last_mm_of_bank = {}
    for i, (j, bank, lo, w) in enumerate(mms):
        first_mm_of_bank.setdefault(bank, i)
        last_mm_of_bank[bank] = i

    nc = bass.Bass()
    pts = nc.dram_tensor("pts", [P, nch * C], fp8, kind="ExternalInput")
    meta = nc.dram_tensor("meta", [P, MW], bf16, kind="ExternalInput")
    outp = nc.dram_tensor("out", [C, S], bf16, kind="ExternalOutput")
    wb_names = {}

    with tile.TileContext(nc) as tc:
        with tc.tile_pool(name="sb", bufs=1) as con, \
             tc.tile_pool(name="ps", bufs=1, space="PSUM") as ps:
            meta_sb = con.tile([P, MW], bf16, tag="meta")
            iota_rep = con.tile([P, OH_W * KB], bf16, tag="iota")
            stage = con.tile([P, S], bf16, tag="stage")
            slabs = [con.tile([P, int(w) * C], fp8, name=f"slab{k}",
                              tag=f"slab{k}") for k, w in enumerate(plan)]
            ohs = [con.tile([P, OH_W * KB], bf16, name=f"oh{k}",
                            tag=f"oh{k}") for k in range(-(-len(mms) // KB))]
            accs = [ps.tile([P, BANK_W], f32, name=f"acc{k}", tag=f"acc{k}")
                    for k in range(n_banks)]

            # meta rides 4th in the issue order: the first three slabs keep
            # the DMA engines saturated while the HWDGE issue chain catches
            # up (meta-2nd left a 117ns transfer gap at slab2), and meta
            # still lands ~5.3us — far ahead of the one-hot builds' needs
            for i, w in enumerate(plan):
                c0 = int(slab_c0[i])
                nc.sync.dma_start(out=slabs[i][:],
                                  in_=pts[:, c0 * C:(c0 + int(w)) * C])
                if i == 2:
                    nc.sync.dma_start(out=meta_sb[:], in_=meta[:])
            if len(plan) <= 2:
                nc.sync.dma_start(out=meta_sb[:], in_=meta[:])

            # compare target for the one-hot build: value w at w*KB+m,
            # built by 16 DVE memsets during the otherwise-idle preamble
            for wv in range(OH_W):
                nc.vector.memset(iota_rep[:, wv * KB:(wv + 1) * KB],
                                 float(wv))

            mstride = meta_sb[:].ap[0][0]
            for m, (j, bank, lo, w) in enumerate(mms):
                b = m // KB
                if m % KB == 0:
                    oh = ohs[b]
                    ostride = oh[:].ap[0][0]
                    istride = iota_rep[:].ap[0][0]
                    # transposed batch layout oh[p, w*KB + m']: every
                    # operand's LAST AP dim is packed stride-1 bf16, which
                    # unlocks the DVE 2x perf mode (is_equal is 2x_1p-only)
                    out_ap = bass.AP(oh[:].tensor, 0,
                                     [[ostride, P], [KB, OH_W], [1, KB]])
                    iota_ap = bass.AP(iota_rep[:].tensor, 0,
                                      [[istride, P], [KB, OH_W], [1, KB]])
                    lid_ap = bass.AP(meta_sb[:].tensor, KB * b,
                                     [[mstride, P], [0, OH_W], [1, KB]])
                    nc.vector.tensor_tensor(
                        out=out_ap, in0=iota_ap, in1=lid_ap,
                        op=mybir.AluOpType.is_equal)
                si = int(slab_of_chunk[j])
                cj = j - int(slab_c0[si])
                ostride = ohs[b][:].ap[0][0]
                rhs_ap = bass.AP(ohs[b][:].tensor, m % KB,
                                 [[ostride, P], [KB, w]])
                # the first matmul touching a bank runs start=True: it
                # zeroes the bank's whole 2KB zero region before its own
                # accumulation, so no separate zero-matmul feed is needed
                nc.tensor.matmul(
                    out=accs[bank][0:C, lo:lo + w],
                    lhsT=slabs[si][:, cj * C:(cj + 1) * C],
                    rhs=rhs_ap,
                    start=(m == first_mm_of_bank[bank]),
                    stop=(m == last_mm_of_bank[bank]))
                for r, (r0, r1) in enumerate(ranges):
                    if last_mm_of_rng.get(r) != m:
                        continue
                    w2 = r1 - r0
                    bank_r, lo_r = pbank(r0)
                    if len(by_completion) > 1 and r == by_completion[-2]:
                        # second tail range: PSUM->SBUF on DVE; its Act-queue
                        # out-DMA is deferred to the final range's emission
                        # point so the Act SEQ reaches the final range's
                        # copy without a 667ns DMACopy slot in front of it
                        nc.vector.tensor_scalar(
                            out=stage[0:C, r0:r0 + w2],
                            in0=accs[bank_r][0:C, lo_r:lo_r + w2],
                            scalar1=0.0, scalar2=None,
                            op0=mybir.AluOpType.add)
                        continue
                    # bulk copies + the final range's copy ride the scalar
                    # engine (idle at the tail, lower PSUM-read sem latency
                    # than DVE); bulk writebacks ride the Pool SWDGE path
                    # (its sem pool is disjoint from the slab DMAs' HWDGE
                    # sems), the last two the by-then-idle SP/Act HWDGE
                    # queues.
                    nc.scalar.activation(
                        out=stage[0:C, r0:r0 + w2],
                        in_=accs[bank_r][0:C, lo_r:lo_r + w2],
                        func=mybir.ActivationFunctionType.Copy)
                    if r == by_completion[-1]:
                        dma = nc.sync.dma_start(out=outp[:, r0:r0 + w2],
                                                in_=stage[0:C, r0:r0 + w2])
                        wb_names[dma.ins.name] = by_completion.index(r)
                        if len(by_completion) > 1:
                            r2 = by_completion[-2]
                            p0, p1 = ranges[r2]
                            dma = nc.scalar.dma_start(
                                out=outp[:, p0:p1],
                                in_=stage[0:C, p0:p1])
                            wb_names[dma.ins.name] = by_completion.index(r2)
                    else:
                        dma = nc.gpsimd.dma_start(out=outp[:, r0:r0 + w2],
                                                  in_=stage[0:C, r0:r0 + w2])
                        wb_names[dma.ins.name] = by_completion.index(r)

    _rank_dma_lanes(nc, wb_names)
    return nc


def _rank_dma_lanes(nc, wb_names):
    """Order the end-barrier's sem-wait walk by expected resolution time.

    Replicates tile_sem_assignment's round-robin DMA lane assignment
    (DMASW lanes for Pool-engine DMAs, DMAHW lanes for the rest, in final
    block order) and ranks each lane by its latest user: slab/meta
    transfers resolve ~stream-end+0.9us, writebacks in completion order
    after. _split_multi_waits sorts each multi-wait by these ranks so the
    barrier walk parks on the true last event instead of serializing
    ~0.7us of already-satisfied waits behind it."""
    import bass_rust as _br
    import concourse.mybir as mybir

    n_wb = max(wb_names.values()) + 1 if wb_names else 0
    sw = hw = 0
    lane_user_rank = {}
    for bb in nc.m.functions[0].blocks:
        for inst in bb.instructions:
            if not isinstance(inst, mybir.InstDMACopy):
                continue
            if inst.engine == mybir.EngineType.Pool:
                lane = f"DMASW{sw % _br.NUM_SWDGE_GLOBAL_SEMS}"
                sw += 1
            else:
                lane = f"DMAHW{hw % _br.NUM_HWDGE_SEMS}"
                hw += 1
            if inst.name in wb_names:
                i = wb_names[inst.name]
                rank = (20 + i if i < n_wb - 2 else
                        40 if i == n_wb - 2 else 50)
            else:
                rank = 10
            lane_user_rank[lane] = max(lane_user_rank.get(lane, 0), rank)
    nc._bev_sem_rank = lane_user_rank
    return nc


def _split_multi_waits(nc):
    """Walrus codegen allows a single sync-wait slot per instruction struct;
    hoist all but the last wait of any multi-wait instruction onto preceding
    single-wait EventSemaphore instructions on the same engine queue.

    The hoisted event-sems serialize at ~50ns each on the queue, so order
    matters for wide waits (the end-of-program barrier waits ~24 sems):
    engine sems (resolved early) walk first, then DMA-lane sems in the
    expected resolution order computed by _rank_dma_lanes."""
    import concourse.mybir as mybir

    sem_rank = getattr(nc, "_bev_sem_rank", {})

    def key(w):
        nm = w.ant_name or ""
        if nm.startswith("DMASW") or nm.startswith("DMAHW"):
            grp = sem_rank.get(nm.split("_")[0], 15)
        else:
            grp = 0
        return (grp, w.wait_value or 0)

    k = 0
    for bb in nc.m.functions[0].blocks:
        new = []
        changed = False
        for inst in bb.instructions:
            si = inst.sync_info
            if si is not None and si.on_wait and len(si.on_wait) > 1:
                waits = sorted(si.on_wait, key=key)
                for w in waits[:-1]:
                    ev = mybir.InstEventSemaphore(
                        name=f"wsplit-{k}", ins=[], outs=[])
                    k += 1
                    ev.engine = inst.engine
                    ev.sync_info = mybir.SyncInfo(on_wait=[w], on_update=[])
                    nc.inst_map[ev.name] = ev
                    new.append(ev)
                si.on_wait = [waits[-1]]
                changed = True
            new.append(inst)
        if changed:
            try:
                bb.instructions = new
            except Exception:
                bb.instructions[:] = new
    return nc


def kernel(feats, img_trans, img_scale, lidar2img):
    from concourse import bass_utils

    feats = np.ascontiguousarray(feats, dtype=np.float32)
    img_trans = np.asarray(img_trans, dtype=np.float32)
    img_scale = np.asarray(img_scale, dtype=np.float32)
    lidar2img = np.asarray(lidar2img, dtype=np.float32)
    B, N, D, H, W, C = feats.shape
    npt = B * N * D * H * W

    flatm, (NX, NY, NZ) = _host_geometry(img_trans, img_scale, lidar2img,
                                         B, N, D, H, W)
    out = np.zeros((B, NZ * C, NX, NY), np.float32)
    ib = flatm >= 0
    if not ib.any():
        return out
    uvox, slot_all = np.unique(flatm[ib], return_inverse=True)
    S = len(uvox)

    # leveled core assignment: each slot's points split as evenly as
    # possible, remainders to the cores currently behind, so per-core
    # cumulative counts never differ by more than 1. Every core's stream
    # is then its own points packed densely in slot order (no per-slot
    # padding), and at any shared chunk boundary the cores sit within a
    # couple of slots of each other — the shared window schedule still
    # covers all of them, with per-core lid columns in meta.
    cnt = np.bincount(slot_all, minlength=S)
    c = np.zeros((N_CORES, S), np.int64)
    tot = np.zeros(N_CORES, np.int64)
    for s in range(S):
        base, r = divmod(int(cnt[s]), N_CORES)
        c[:, s] = base
        if r:
            c[np.argsort(tot, kind='stable')[:r], s] += 1
        tot += c[:, s]
    T = int(tot.max())
    nch = -(-T // P)
    Mp = nch * P

    startk = np.zeros((N_CORES, S), np.int64)
    startk[:, 1:] = np.cumsum(c, axis=1)[:, :-1]
    slots8_asc = np.full((N_CORES, Mp), -1, np.int64)
    for k in range(N_CORES):
        slots8_asc[k, :int(tot[k])] = np.repeat(np.arange(S), c[k])
    # process chunks in descending-slot order: the sparse high-slot tail
    # (many columns finishing at once) streams first and its writebacks
    # overlap the stream; the final chunks touch only the lowest slots,
    # so the post-stream tail copies almost nothing
    slots8 = slots8_asc.reshape(N_CORES, nch, P)[:, ::-1] \
        .reshape(N_CORES, Mp)

    fins = _pick_fins(slots8, nch, _slab_plan(nch))
    mms, lids8, n_mmp, last_mm_of_bank = _build_schedule(slots8, nch, S,
                                                         fins)

    # per-point core + stream position (ascending slot-sorted layout)
    srt = np.argsort(slot_all, kind='stable')
    ss = slot_all[srt]
    starts = np.zeros(S, np.int64)
    starts[1:] = np.cumsum(cnt)[:-1]
    rank = np.arange(len(ss)) - starts[ss]          # rank within slot
    cums = np.zeros((N_CORES + 1, S), np.int64)
    cums[1:] = np.cumsum(c, axis=0)
    core_of = np.empty(len(ss), np.int64)
    qrank = np.empty(len(ss), np.int64)
    for k in range(N_CORES):
        sel = (rank >= cums[k][ss]) & (rank < cums[k + 1][ss])
        core_of[sel] = k
        qrank[sel] = rank[sel] - cums[k][ss][sel]
    lpos_asc = startk[core_of, ss] + qrank
    feats_ib = feats.reshape(npt, C)[ib][srt]       # fp32, slot-sorted

    # ---- fp8 e3m4 quantization with per-(core,slot) error feedback ----
    # q_i = rnd(x_i + carry); the device's exact fp32 partial sum of a
    # chain then equals the true sum minus only the final carry.
    x_asc = np.zeros((N_CORES, Mp, C), np.float32)
    x_asc[core_of, lpos_asc] = feats_ib
    q_asc = np.zeros((N_CORES, Mp, C), FP8)
    carry = np.zeros((N_CORES, S, C), np.float32)
    for r in range(int(c.max())):
        k_idx, s_idx = np.nonzero(c > r)
        pos = startk[k_idx, s_idx] + r
        t = x_asc[k_idx, pos] + carry[k_idx, s_idx]
        q8 = t.astype(FP8)
        q_asc[k_idx, pos] = q8
        carry[k_idx, s_idx] = t - q8.astype(np.float32)

    MW = max(n_mmp, 256)
    metas = []
    for k in range(N_CORES):
        mnp = np.full((P, MW), -1.0, BF16)
        mnp[:, :n_mmp] = lids8[k].astype(BF16)
        metas.append(mnp)

    nc = _build_bass(nch, n_mmp, mms, last_mm_of_bank, S, C, fins)
    _split_multi_waits(nc)

    in_maps = []
    for core in range(N_CORES):
        # chunk-reverse (descending-slot order) + chunk-transpose
        q_rev = q_asc[core].reshape(nch, P, C)[::-1]
        pts_c = np.ascontiguousarray(
            q_rev.transpose(1, 0, 2).reshape(P, nch * C))
        in_maps.append({"pts": pts_c, "meta": metas[core]})

    if bool(int(os.environ.get("BEV_TIMELINE", "0"))):
        from concourse.timeline_sim import TimelineSim
        t_ns = TimelineSim(nc).simulate()
        print(f"HW exec time: {t_ns:.0f} ns")
    res = bass_utils.run_bass_kernel_spmd(
        nc, in_maps, core_ids=list(range(N_CORES)))

    total = np.zeros((C, S), np.float64)
    for r in res.results:
        total += np.asarray(r["out"], dtype=np.float64)
    total = total.astype(np.float32)

    gsz = NZ * NX * NY
    b_u = uvox // gsz
    r_u = uvox % gsz
    z_u = r_u // (NX * NY)
    xy_u = r_u % (NX * NY)
    ov = out.reshape(B, NZ, C, NX * NY)
    ov[b_u, z_u, :, xy_u] = total.T
    return out


# revision 47
# speedup vs baseline: 1.0290x; 1.0290x over previous
"""BEV-pool (lift-splat-shoot scatter-sum) Trainium2 Bass kernel, v4.

Pipeline
--------
Host (numpy, index math + quantization only):
  * mirror the reference geometry in float32 to voxelize every frustum
    point (value-identical to the jax/CPU reference on in-bounds points)
  * compact the ~2k occupied voxels into a dense slot space; split each
    voxel's points across the 8 cores with leveled remainders so per-core
    cumulative counts never drift by more than 1: every core's stream is
    its own points packed densely in slot order with NO per-slot padding,
    yet one shared SPMD window schedule covers all cores (per-core lid
    columns carry the differences)
  * quantize the point features to fp8 e3m4 with ERROR FEEDBACK along
    each (core, voxel) chain: q_i = rnd(x_i + carry_{i-1}). The device
    sums each chain exactly in fp32 PSUM, so a voxel partial-sum carries
    only the FINAL carry (~one e3m4 ulp) instead of sqrt(k) accumulated
    ulps -> ~0.33% rel err at HALF the bf16 stream bytes
  * lay each core's points out slot-sorted and chunk-transposed in DRAM
    ([128, nch*80] fp8) so the device streams them with plain wide
    contiguous DMAs at the full DMA-fleet rate; chunk order is
    descending-slot so the final chunks touch only the lowest slots and
    the post-stream writeback tail is minimal
  * bake the per-matmul window-relative slot ids (bf16) into a small
    meta tensor

Device (per core, Bass/Tile):
  * slab0 is issued before the short meta transfer so the DMA engines
    are never idle during the HWDGE issue-chain fill; ~9us of fp8 slab
    transfers then stream back-to-back on the sync-engine HWDGE queue
  * one-hot rhs per matmul, built 32-matmuls-per-DVE-instruction in a
    TRANSPOSED batch layout oh[p, w*KB + m]: every operand's last AP dim
    is packed stride-1 bf16, so the DVE runs its 2x perf mode
    (~10ns/matmul); iota_rep (compare target, value w at w*KB+m) is 16
    DVE memsets during the preamble
  * matmuls use the ACTUAL window span (mean ~7 cols), keeping PE ~2us;
    the first matmul touching each PSUM bank runs start=True, zeroing
    the bank's 2KB zero region (no separate zero-matmul feed)
  * the window schedule is clipped at two forced boundaries fins =
    (max slot of the last slab, max slot of the last two slabs), and
    those two tail column ranges accumulate in their own spare PSUM
    banks: Tile tracks PSUM WAR hazards per bank, so the earlier
    ranges' PSUM-read copies never stall the tail slabs' matmuls
  * writeback overlaps the stream: each column range is copied
    PSUM->SBUF the moment its last matmul retires (scalar engine for
    the bulk ranges, DVE for the two tail ranges so they don't queue
    behind a bulk copy) and DMA'd out — bulk ranges on the Pool SWDGE
    queue (its sem pool is disjoint from the slab DMAs' HWDGE sems),
    the final two on the by-then-idle sync/scalar HWDGE queues
  * a post-pass orders the end-of-program barrier's split sem-wait walk
    by expected resolution time (slab lanes, then writeback lanes in
    completion order), so the walk parks on the true last event instead
    of serializing ~0.7us of satisfied waits behind it

Host combine: sum the 8 aligned [80, 2044] bf16 partials in fp64,
scatter the compact slot rows into the zeros output grid.

A post-pass splits multi-wait instructions into single-wait
EventSemaphores (this walrus build accepts only one sync-wait slot per
instruction struct).
"""

import os
import numpy as np
import ml_dtypes

BF16 = ml_dtypes.bfloat16
FP8 = ml_dtypes.float8_e3m4

# ---- problem constants (from the reference nn.Module) ----
IMAGE_SIZE = (256, 704)
FEATURE_SIZE = (32, 88)
XBOUND = (-54.0, 54.0, 0.3)
YBOUND = (-54.0, 54.0, 0.3)
ZBOUND = (-10.0, 10.0, 20.0)
DBOUND = (1.0, 60.0, 1.0)

N_CORES = 8
P = 128          # points per chunk / matmul contraction dim
OH_W = 16        # one-hot window width (max moving cols per matmul)
BANK_W = 512     # PSUM bank width in fp32
KB = 32          # matmuls per batched DVE is_equal
CS = 32          # chunks per stream slab DMA


def _host_geometry(img_trans, img_scale, lidar2img, B, N, D, H, W):
    """float32 numpy mirror of the reference get_geometry + voxelize."""
    dx = np.array([XBOUND[2], YBOUND[2], ZBOUND[2]], np.float32)
    bx = np.array([XBOUND[0] + XBOUND[2] / 2.0,
                   YBOUND[0] + YBOUND[2] / 2.0,
                   ZBOUND[0] + ZBOUND[2] / 2.0], np.float32)
    nx = [int((b[1] - b[0]) / b[2]) for b in (XBOUND, YBOUND, ZBOUND)]
    NX, NY, NZ = nx

    iH, iW = IMAGE_SIZE
    fH, fW = FEATURE_SIZE
    ds = np.arange(DBOUND[0], DBOUND[1], DBOUND[2], dtype=np.float32)
    xs = np.linspace(0.0, iW - 1, fW, dtype=np.float32)
    ys = np.linspace(0.0, iH - 1, fH, dtype=np.float32)
    assert ds.shape[0] == D and fH == H and fW == W

    fr = np.stack([
        np.broadcast_to(xs[None, None, :], (D, H, W)),
        np.broadcast_to(ys[None, :, None], (D, H, W)),
        np.broadcast_to(ds[:, None, None], (D, H, W)),
    ], axis=-1).astype(np.float32)                       # [D,H,W,3]

    pts = fr[None, None] + img_trans[:, :, None, None, None, :]
    d = pts[..., 2:3]
    xy = pts[..., :2] / img_scale[:, :, None, None, None, None]
    p4 = np.concatenate([xy * d, d, np.ones_like(d)], axis=-1)
    img2lidar = np.linalg.inv(lidar2img)
    geom = np.einsum('bnij,bndhwj->bndhwi', img2lidar, p4)[..., :3]
    geom = geom.astype(np.float32)
    vox = ((geom - (bx - dx / 2.0)) / dx).astype(np.int32)  # trunc toward 0
    mask = ((vox[..., 0] >= 0) & (vox[..., 0] < NX)
            & (vox[..., 1] >= 0) & (vox[..., 1] < NY)
            & (vox[..., 2] >= 0) & (vox[..., 2] < NZ))
    flat = (vox[..., 2] * NX + vox[..., 0]) * NY + vox[..., 1]
    flat = flat + np.arange(B, dtype=np.int32)[:, None, None, None, None] \
        * (NZ * NX * NY)
    flatm = np.where(mask, flat, -1).reshape(-1)
    return flatm, (NX, NY, NZ)


def _pick_fins(slots8, nch, plan):
    """Forced writeback-range boundaries: just above the highest slot the
    last slab (and the last two slabs) touch, so the bulk of the grid
    writes back while the tail slabs' DMA-sem propagations are still in
    flight and only two small copies + DMAs trail the stream."""
    fins = []
    for k in (1, 2):
        w = sum(plan[-k:])
        tail = slots8[:, (nch - w) * P:]
        real = tail[tail >= 0]
        f = int(real.max()) + 1 if len(real) else 1
        fins.append(min(f, BANK_W))
    fins = sorted(set(f for f in fins if 0 < f < BANK_W))
    return fins


def _build_schedule(slots8, nch, S, fins):
    """Per-matmul windows over the per-core slot-sorted point layouts.

    slots8: [N_CORES, nch*P] slot id per stream position (-1 = zero pad).
    The window schedule is shared across cores (SPMD program); each core
    gets its own lid columns. Returns (mm list of (chunk, bank, col_lo,
    w_actual), lids [N_CORES, P, n_mmp] f32, n_mmp, last_mm_of_bank).
    w_actual is the true span of occupied slots in the window, so the
    matmul only streams the PSUM columns that can receive points.
    Windows are clipped at the forced fins (as at bank boundaries) so no
    matmul straddles a writeback range edge.
    """
    mms = []
    lid_cols = [[] for _ in range(N_CORES)]
    for j in range(nch):
        sl8 = slots8[:, j * P:(j + 1) * P]
        real = sl8[sl8 >= 0]
        if len(real) == 0:
            continue
        cur = int(real.min())
        while True:
            bank = cur // BANK_W
            wend = min(cur + OH_W, (bank + 1) * BANK_W, S)
            for f in fins:
                if cur < f:
                    wend = min(wend, f)
                    break
            inw = real[(real >= cur) & (real < wend)]
            w_act = int(inw.max()) - cur + 1
            for k in range(N_CORES):
                lid_cols[k].append(
                    np.where((sl8[k] >= cur) & (sl8[k] < wend),
                             sl8[k] - cur, -1.0).astype(np.float32))
            mms.append((j, bank, cur - bank * BANK_W, w_act))
            nxt = real[real >= wend]
            if len(nxt) == 0:
                break
            cur = int(nxt.min())
    n_mm = len(mms)
    n_mmp = -(-n_mm // KB) * KB
    lids = np.full((N_CORES, P, n_mmp), -1.0, np.float32)
    if n_mm:
        for k in range(N_CORES):
            lids[k, :, :n_mm] = np.stack(lid_cols[k], axis=1)
    last_mm_of_bank = {}
    for i, (j, bank, lo, w) in enumerate(mms):
        last_mm_of_bank[bank] = i
    return mms, lids, n_mmp, last_mm_of_bank


def _wb_ranges(mms, S, fins):
    """Writeback column ranges + the last matmul touching each; ranges are
    written back the moment their last matmul retires. Range edges are
    bank boundaries plus the forced fins."""
    bounds = sorted({0} | {min(f, S) for f in fins} |
                    {b for b in range(BANK_W, S, BANK_W)} | {S})
    ranges = list(zip(bounds[:-1], bounds[1:]))
    last_mm_of_rng = {}
    for i, (j, bank, lo, w) in enumerate(mms):
        c0, c1 = bank * BANK_W + lo, bank * BANK_W + lo + w
        for r, (r0, r1) in enumerate(ranges):
            if c0 < r1 and c1 > r0:
                last_mm_of_rng[r] = i
    by_completion = sorted(last_mm_of_rng, key=lambda r: last_mm_of_rng[r])
    return ranges, last_mm_of_rng, by_completion


def _slab_plan(nch):
    """Chunk counts per slab DMA: CS-sized middles, shrinking tail (short
    post-stream dependency chain; last slab >=12 chunks keeps its
    per-descriptor bytes above the 512B small-transfer penalty)."""
    tail = [c for c in (16, 12) if nch > 48]
    mid = nch - sum(tail)
    plan = [CS] * (mid // CS)
    if mid % CS:
        plan.append(mid % CS)
    plan += tail
    assert sum(plan) == nch and plan
    return plan


def _build_bass(nch, n_mmp, mms, last_mm_of_bank, S, C, fins):
    import concourse.bass as bass
    import concourse.mybir as mybir
    import concourse.tile as tile
    from concourse.instruction_name_ordered_set import InstructionNameOrderedSet

    f32 = mybir.dt.float32
    bf16 = mybir.dt.bfloat16
    fp8 = mybir.dt.float8e3
    MW = max(n_mmp, 256)                    # >=512B/partition meta transfer
    plan = _slab_plan(nch)
    slab_of_chunk = np.repeat(np.arange(len(plan)), plan)
    slab_c0 = np.concatenate([[0], np.cumsum(plan)[:-1]])

    ranges, last_mm_of_rng, by_completion = _wb_ranges(mms, S, fins)

    # Tile tracks PSUM WAR hazards at bank granularity: an act-copy
    # reading a bank's finished column range would serialize every LATER
    # matmul into the same bank behind it (+~1us on the tail). Give each
    # forced tail range [0,fins[0]) / [fins[0],fins[1]) its own spare
    # PSUM bank so the tail matmuls and the earlier ranges' reads never
    # share a bank.
    n_grid = -(-S // BANK_W)

    def pbank(c0):
        for i, f in enumerate(fins):
            if c0 < f:
                base = fins[i - 1] if i else 0
                return n_grid + i, c0 - base
        return c0 // BANK_W, c0 % BANK_W

    n_banks = n_grid + len(fins)
    assert n_banks <= 8
    mms = [(j, *pbank(bank * BANK_W + lo), w) for (j, bank, lo, w) in mms]
    first_mm_of_bank = {}
    last_mm_of_bank = {}
    for i, (j, bank, lo, w) in enumerate(mms):
        first_mm_of_bank.setdefault(bank, i)
        last_mm_of_bank[bank] = i

    nc = bass.Bass()
    pts = nc.dram_tensor("pts", [P, nch * C], fp8, kind="ExternalInput")
    meta = nc.dram_tensor("meta", [P, MW], bf16, kind="ExternalInput")
    outp = nc.dram_tensor("out", [C, S], bf16, kind="ExternalOutput")
    wb_names = {}

    with tile.TileContext(nc) as tc:
        with tc.tile_pool(name="sb", bufs=1) as con, \
             tc.tile_pool(name="ps", bufs=1, space="PSUM") as ps:
            meta_sb = con.tile([P, MW], bf16, tag="meta")
            iota_rep = con.tile([P, OH_W * KB], bf16, tag="iota")
            stage = con.tile([P, S], bf16, tag="stage")
            slabs = [con.tile([P, int(w) * C], fp8, name=f"slab{k}",
                              tag=f"slab{k}") for k, w in enumerate(plan)]
            ohs = [con.tile([P, OH_W * KB], bf16, name=f"oh{k}",
                            tag=f"oh{k}") for k in range(-(-len(mms) // KB))]
            accs = [ps.tile([P, BANK_W], f32, name=f"acc{k}", tag=f"acc{k}")
                    for k in range(n_banks)]

            # meta rides 4th in the issue order: the first three slabs keep
            # the DMA engines saturated while the HWDGE issue chain catches
            # up (meta-2nd left a 117ns transfer gap at slab2), and meta
            # still lands ~5.3us — far ahead of the one-hot builds' needs
            for i, w in enumerate(plan):
                c0 = int(slab_c0[i])
                nc.sync.dma_start(out=slabs[i][:],
                                  in_=pts[:, c0 * C:(c0 + int(w)) * C])
                if i == 2:
                    nc.sync.dma_start(out=meta_sb[:], in_=meta[:])
            if len(plan) <= 2:
                nc.sync.dma_start(out=meta_sb[:], in_=meta[:])

            # compare target for the one-hot build: value w at w*KB+m,
            # built by 16 DVE memsets during the otherwise-idle preamble
            for wv in range(OH_W):
                nc.vector.memset(iota_rep[:, wv * KB:(wv + 1) * KB],
                                 float(wv))

            mstride = meta_sb[:].ap[0][0]
            for m, (j, bank, lo, w) in enumerate(mms):
                b = m // KB
                if m % KB == 0:
                    oh = ohs[b]
                    ostride = oh[:].ap[0][0]
                    istride = iota_rep[:].ap[0][0]
                    # transposed batch layout oh[p, w*KB + m']: every
                    # operand's LAST AP dim is packed stride-1 bf16, which
                    # unlocks the DVE 2x perf mode (is_equal is 2x_1p-only)
                    out_ap = bass.AP(oh[:].tensor, 0,
                                     [[ostride, P], [KB, OH_W], [1, KB]])
                    iota_ap = bass.AP(iota_rep[:].tensor, 0,
                                      [[istride, P], [KB, OH_W], [1, KB]])
                    lid_ap = bass.AP(meta_sb[:].tensor, KB * b,
                                     [[mstride, P], [0, OH_W], [1, KB]])
                    nc.vector.tensor_tensor(
                        out=out_ap, in0=iota_ap, in1=lid_ap,
                        op=mybir.AluOpType.is_equal)
                si = int(slab_of_chunk[j])
                cj = j - int(slab_c0[si])
                ostride = ohs[b][:].ap[0][0]
                rhs_ap = bass.AP(ohs[b][:].tensor, m % KB,
                                 [[ostride, P], [KB, w]])
                # the first matmul touching a bank runs start=True: it
                # zeroes the bank's whole 2KB zero region before its own
                # accumulation, so no separate zero-matmul feed is needed
                nc.tensor.matmul(
                    out=accs[bank][0:C, lo:lo + w],
                    lhsT=slabs[si][:, cj * C:(cj + 1) * C],
                    rhs=rhs_ap,
                    start=(m == first_mm_of_bank[bank]),
                    stop=(m == last_mm_of_bank[bank]))
                for r, (r0, r1) in enumerate(ranges):
                    if last_mm_of_rng.get(r) != m:
                        continue
                    w2 = r1 - r0
                    bank_r, lo_r = pbank(r0)
                    if len(by_completion) > 1 and r == by_completion[-2]:
                        # second tail range: PSUM->SBUF on DVE; its Act-queue
                        # out-DMA is deferred to the final range's emission
                        # point so the Act SEQ reaches the final range's
                        # copy without a 667ns DMACopy slot in front of it
                        nc.vector.tensor_scalar(
                            out=stage[0:C, r0:r0 + w2],
                            in0=accs[bank_r][0:C, lo_r:lo_r + w2],
                            scalar1=0.0, scalar2=None,
                            op0=mybir.AluOpType.add)
                        continue
                    # bulk copies + the final range's copy ride the scalar
                    # engine (idle at the tail, lower PSUM-read sem latency
                    # than DVE); bulk writebacks ride the Pool SWDGE path
                    # (its sem pool is disjoint from the slab DMAs' HWDGE
                    # sems), the last two the by-then-idle SP/Act HWDGE
                    # queues.
                    cp = nc.scalar.activation(
                        out=stage[0:C, r0:r0 + w2],
                        in_=accs[bank_r][0:C, lo_r:lo_r + w2],
                        func=mybir.ActivationFunctionType.Copy)
                    if r == by_completion[-1]:
                        dma = nc.sync.dma_start(out=outp[:, r0:r0 + w2],
                                                in_=stage[0:C, r0:r0 + w2])
                        wb_names[dma.ins.name] = by_completion.index(r)
                        if len(by_completion) > 1:
                            r2 = by_completion[-2]
                            p0, p1 = ranges[r2]
                            dma = nc.scalar.dma_start(
                                out=outp[:, p0:p1],
                                in_=stage[0:C, p0:p1])
                            wb_names[dma.ins.name] = by_completion.index(r2)
                            # Tile's scheduler would otherwise hoist this
                            # ready DMACopy ahead of the final range's act
                            # on the Act SEQ, parking the critical copy
                            # behind a 667ns DMA-issue slot
                            deps = InstructionNameOrderedSet()
                            deps.add(cp.ins.name)
                            dma.ins.add_nosync_dependencies_from(deps)
                    else:
                        dma = nc.gpsimd.dma_start(out=outp[:, r0:r0 + w2],
                                                  in_=stage[0:C, r0:r0 + w2])
                        wb_names[dma.ins.name] = by_completion.index(r)

    _rank_dma_lanes(nc, wb_names)
    return nc


def _rank_dma_lanes(nc, wb_names):
    """Order the end-barrier's sem-wait walk by expected resolution time.

    Replicates tile_sem_assignment's round-robin DMA lane assignment
    (DMASW lanes for Pool-engine DMAs, DMAHW lanes for the rest, in final
    block order) and ranks each lane by its latest user: slab/meta
    transfers resolve ~stream-end+0.9us, writebacks in completion order
    after. _split_multi_waits sorts each multi-wait by these ranks so the
    barrier walk parks on the true last event instead of serializing
    ~0.7us of already-satisfied waits behind it."""
    import bass_rust as _br
    import concourse.mybir as mybir

    n_wb = max(wb_names.values()) + 1 if wb_names else 0
    sw = hw = 0
    lane_user_rank = {}
    for bb in nc.m.functions[0].blocks:
        for inst in bb.instructions:
            if not isinstance(inst, mybir.InstDMACopy):
                continue
            if inst.engine == mybir.EngineType.Pool:
                lane = f"DMASW{sw % _br.NUM_SWDGE_GLOBAL_SEMS}"
                sw += 1
            else:
                lane = f"DMAHW{hw % _br.NUM_HWDGE_SEMS}"
                hw += 1
            if inst.name in wb_names:
                i = wb_names[inst.name]
                rank = (20 + i if i < n_wb - 2 else
                        40 if i == n_wb - 2 else 50)
            else:
                rank = 10
            lane_user_rank[lane] = max(lane_user_rank.get(lane, 0), rank)
    nc._bev_sem_rank = lane_user_rank
    return nc


def _split_multi_waits(nc):
    """Walrus codegen allows a single sync-wait slot per instruction struct;
    hoist all but the last wait of any multi-wait instruction onto preceding
    single-wait EventSemaphore instructions on the same engine queue.

    The hoisted event-sems serialize at ~50ns each on the queue, so order
    matters for wide waits (the end-of-program barrier waits ~24 sems):
    engine sems (resolved early) walk first, then DMA-lane sems in the
    expected resolution order computed by _rank_dma_lanes."""
    import concourse.mybir as mybir

    sem_rank = getattr(nc, "_bev_sem_rank", {})

    def key(w):
        nm = w.ant_name or ""
        if nm.startswith("DMASW") or nm.startswith("DMAHW"):
            grp = sem_rank.get(nm.split("_")[0], 15)
        else:
            grp = 0
        return (grp, w.wait_value or 0)

    k = 0
    for bb in nc.m.functions[0].blocks:
        new = []
        changed = False
        for inst in bb.instructions:
            si = inst.sync_info
            if si is not None and si.on_wait and len(si.on_wait) > 1:
                waits = sorted(si.on_wait, key=key)
                for w in waits[:-1]:
                    ev = mybir.InstEventSemaphore(
                        name=f"wsplit-{k}", ins=[], outs=[])
                    k += 1
                    ev.engine = inst.engine
                    ev.sync_info = mybir.SyncInfo(on_wait=[w], on_update=[])
                    nc.inst_map[ev.name] = ev
                    new.append(ev)
                si.on_wait = [waits[-1]]
                changed = True
            new.append(inst)
        if changed:
            try:
                bb.instructions = new
            except Exception:
                bb.instructions[:] = new
    return nc


def kernel(feats, img_trans, img_scale, lidar2img):
    from concourse import bass_utils

    feats = np.ascontiguousarray(feats, dtype=np.float32)
    img_trans = np.asarray(img_trans, dtype=np.float32)
    img_scale = np.asarray(img_scale, dtype=np.float32)
    lidar2img = np.asarray(lidar2img, dtype=np.float32)
    B, N, D, H, W, C = feats.shape
    npt = B * N * D * H * W

    flatm, (NX, NY, NZ) = _host_geometry(img_trans, img_scale, lidar2img,
                                         B, N, D, H, W)
    out = np.zeros((B, NZ * C, NX, NY), np.float32)
    ib = flatm >= 0
    if not ib.any():
        return out
    uvox, slot_all = np.unique(flatm[ib], return_inverse=True)
    S = len(uvox)

    # leveled core assignment: each slot's points split as evenly as
    # possible, remainders to the cores currently behind, so per-core
    # cumulative counts never differ by more than 1. Every core's stream
    # is then its own points packed densely in slot order (no per-slot
    # padding), and at any shared chunk boundary the cores sit within a
    # couple of slots of each other — the shared window schedule still
    # covers all of them, with per-core lid columns in meta.
    cnt = np.bincount(slot_all, minlength=S)
    c = np.zeros((N_CORES, S), np.int64)
    tot = np.zeros(N_CORES, np.int64)
    for s in range(S):
        base, r = divmod(int(cnt[s]), N_CORES)
        c[:, s] = base
        if r:
            c[np.argsort(tot, kind='stable')[:r], s] += 1
        tot += c[:, s]
    T = int(tot.max())
    nch = -(-T // P)
    Mp = nch * P

    startk = np.zeros((N_CORES, S), np.int64)
    startk[:, 1:] = np.cumsum(c, axis=1)[:, :-1]
    slots8_asc = np.full((N_CORES, Mp), -1, np.int64)
    for k in range(N_CORES):
        slots8_asc[k, :int(tot[k])] = np.repeat(np.arange(S), c[k])
    # process chunks in descending-slot order: the sparse high-slot tail
    # (many columns finishing at once) streams first and its writebacks
    # overlap the stream; the final chunks touch only the lowest slots,
    # so the post-stream tail copies almost nothing
    slots8 = slots8_asc.reshape(N_CORES, nch, P)[:, ::-1] \
        .reshape(N_CORES, Mp)

    fins = _pick_fins(slots8, nch, _slab_plan(nch))
    mms, lids8, n_mmp, last_mm_of_bank = _build_schedule(slots8, nch, S,
                                                         fins)

    # per-point core + stream position (ascending slot-sorted layout)
    srt = np.argsort(slot_all, kind='stable')
    ss = slot_all[srt]
    starts = np.zeros(S, np.int64)
    starts[1:] = np.cumsum(cnt)[:-1]
    rank = np.arange(len(ss)) - starts[ss]          # rank within slot
    cums = np.zeros((N_CORES + 1, S), np.int64)
    cums[1:] = np.cumsum(c, axis=0)
    core_of = np.empty(len(ss), np.int64)
    qrank = np.empty(len(ss), np.int64)
    for k in range(N_CORES):
        sel = (rank >= cums[k][ss]) & (rank < cums[k + 1][ss])
        core_of[sel] = k
        qrank[sel] = rank[sel] - cums[k][ss][sel]
    lpos_asc = startk[core_of, ss] + qrank
    feats_ib = feats.reshape(npt, C)[ib][srt]       # fp32, slot-sorted

    # ---- fp8 e3m4 quantization with per-(core,slot) error feedback ----
    # q_i = rnd(x_i + carry); the device's exact fp32 partial sum of a
    # chain then equals the true sum minus only the final carry.
    x_asc = np.zeros((N_CORES, Mp, C), np.float32)
    x_asc[core_of, lpos_asc] = feats_ib
    q_asc = np.zeros((N_CORES, Mp, C), FP8)
    carry = np.zeros((N_CORES, S, C), np.float32)
    for r in range(int(c.max())):
        k_idx, s_idx = np.nonzero(c > r)
        pos = startk[k_idx, s_idx] + r
        t = x_asc[k_idx, pos] + carry[k_idx, s_idx]
        q8 = t.astype(FP8)
        q_asc[k_idx, pos] = q8
        carry[k_idx, s_idx] = t - q8.astype(np.float32)

    MW = max(n_mmp, 256)
    metas = []
    for k in range(N_CORES):
        mnp = np.full((P, MW), -1.0, BF16)
        mnp[:, :n_mmp] = lids8[k].astype(BF16)
        metas.append(mnp)

    nc = _build_bass(nch, n_mmp, mms, last_mm_of_bank, S, C, fins)
    _split_multi_waits(nc)

    in_maps = []
    for core in range(N_CORES):
        # chunk-reverse (descending-slot order) + chunk-transpose
        q_rev = q_asc[core].reshape(nch, P, C)[::-1]
        pts_c = np.ascontiguousarray(
            q_rev.transpose(1, 0, 2).reshape(P, nch * C))
        in_maps.append({"pts": pts_c, "meta": metas[core]})

    if bool(int(os.environ.get("BEV_TIMELINE", "0"))):
        from concourse.timeline_sim import TimelineSim
        t_ns = TimelineSim(nc).simulate()
        print(f"HW exec time: {t_ns:.0f} ns")
    res = bass_utils.run_bass_kernel_spmd(
        nc, in_maps, core_ids=list(range(N_CORES)))

    total = np.zeros((C, S), np.float64)
    for r in res.results:
        total += np.asarray(r["out"], dtype=np.float64)
    total = total.astype(np.float32)

    gsz = NZ * NX * NY
    b_u = uvox // gsz
    r_u = uvox % gsz
    z_u = r_u // (NX * NY)
    xy_u = r_u % (NX * NY)
    ov = out.reshape(B, NZ, C, NX * NY)
    ov[b_u, z_u, :, xy_u] = total.T
    return out
